# revision 29
# baseline (speedup 1.0000x reference)
"""Trainium2 Bass kernel for nn_Attention_21208548508357.

Math note: the reference module's einsum is `'bhij,bihd->bihd'` — the value
tensor is indexed with the *query* position `i`, so `j` (the key position)
appears only in the softmax matrix. The einsum therefore reduces to
`v[b,i,h,d] * sum_j att[b,h,i,j]`, and softmax rows sum to exactly 1, so the
whole attention block is the identity on `v`:

    out = (x @ W_v + b_v) @ W_proj + b_proj
        = x @ (W_v @ W_proj) + (b_v @ W_proj + b_proj)

where W_v = W_attn[:, 2E:3E], b_v = b_attn[2E:3E].  The device kernel runs
the token-sharded GEMM `out = x @ W_fused + b_fused` SPMD on 8 NeuronCores
(512 tokens per core), with the tiny 768x768 weight-fold done on host.

Device layout (per core):
  xT  [768, 512]  bf16  — x shard transposed (contraction dim on partitions)
  w   [768, 768]  bf16  — fused weight
  bb  [128, 768]  bf16  — fused bias broadcast to all partitions
  out [512, 768]  bf16  — host upcasts to f32

Structure: PE stationary = 128x128 xT tile, moving = w rows, fp32 PSUM
accumulate over 6 k-tiles; column split 512+256 along PSUM banks.  Weights
and outputs ride the SP HWDGE ring, x chunks the ACT ring.  Token blocks
0/1 chase the arriving chunks and close early, 2/3 backfill, so the DVE
bias-add (fused with the f32->bf16 PSUM->SBUF copy) and the output DMAs
overlap the tail of the matmul stream.  Raw bass (no Tile) — each wait is a
standalone InstEventSemaphore since this walrus build rejects multi-wait
instructions, and every DMA chunk gets its own semaphore so no DMA
completion-order assumptions are needed.
"""

import numpy as np
import sys

if "/opt/trn_rl_repo" not in sys.path:
    sys.path.insert(0, "/opt/trn_rl_repo")

import ml_dtypes
import concourse.bass as bass
import concourse.mybir as mybir
from concourse.bass_utils import run_bass_kernel_spmd

N_CORES = 8
B, S, E = 2, 2048, 768
TOKENS = B * S                    # 4096
TPC = TOKENS // N_CORES           # 512 tokens per core
KT = E // 128                     # 6 contraction tiles of 128
TB = TPC // 128                   # 4 token blocks of 128 per core

BF16 = mybir.dt.bfloat16
F32 = mybir.dt.float32

TRACE = False      # test.py flips this to profile
LAST = None        # last BassKernelResults when TRACE

_nc_cache = None


def _build():
    nc = bass.Bass()
    xT = nc.declare_dram_parameter("xT", [E, TPC], BF16, isOutput=False)
    w = nc.declare_dram_parameter("w", [E, E], BF16, isOutput=False)
    bb = nc.declare_dram_parameter("bb", [128, E], BF16, isOutput=False)
    out = nc.declare_dram_parameter("out", [TPC, E], BF16, isOutput=True)

    with bass.ExitStack() as ctx:
        w_sb = [ctx.enter_context(nc.sbuf_tensor(f"w_sb{k}", [128, E], BF16))
                for k in range(KT)]
        x_sb = [ctx.enter_context(nc.sbuf_tensor(f"x_sb{k}", [128, TPC], BF16))
                for k in range(KT)]
        b_sb = ctx.enter_context(nc.sbuf_tensor("b_sb", [128, E], BF16))
        o_sb = [ctx.enter_context(nc.sbuf_tensor(f"o_sb{t}", [128, E], BF16))
                for t in range(TB)]
        # one PSUM bank (2KB) per tensor: a = f[0:512], b = f[512:768]
        ps_a = [ctx.enter_context(nc.psum_tensor(f"ps_a{t}", [128, 512], F32))
                for t in range(TB)]
        ps_b = [ctx.enter_context(nc.psum_tensor(f"ps_b{t}", [128, 512], F32))
                for t in range(TB)]

        w_sem = [ctx.enter_context(nc.semaphore(f"w_sem{k}")) for k in range(KT)]
        w0h_sem = ctx.enter_context(nc.semaphore("w0h_sem"))
        x0b_sem = ctx.enter_context(nc.semaphore("x0b_sem"))
        x_sem = [ctx.enter_context(nc.semaphore(f"x_sem{k}")) for k in range(KT)]
        bb_sem = ctx.enter_context(nc.semaphore("bb_sem"))
        pe_sem = ctx.enter_context(nc.semaphore("pe_sem"))
        cp_sem = ctx.enter_context(nc.semaphore("cp_sem"))
        out_sem = ctx.enter_context(nc.semaphore("out_sem"))
        block = ctx.enter_context(nc.Block())

        # Column-group close order: (tb, half).  Each entry closes its fp32
        # accumulation independently; the DVE bias-add and output DMA for a
        # group run while later groups are still accumulating on the PE.
        CLOSES = [(0, 0), (1, 0), (0, 1), (1, 1), (2, 0), (2, 1), (3, 0), (3, 1)]

        # SP HWDGE ring: weights (w0 split in half so the first matmul can
        # start sooner), the broadcast bias, then the output DMAs (SP's DGE
        # latency is lower than ACT's and its ring is idle by output time).
        @block.sync
        def _(sync):
            sync.dma_start(out=w_sb[0][:, 512:768], in_=w[0:128, 512:768]
                           ).then_inc(w0h_sem, 16)
            sync.dma_start(out=w_sb[0][:, 0:512], in_=w[0:128, 0:512]
                           ).then_inc(w_sem[0], 16)
            for k in range(1, KT):
                sync.dma_start(out=w_sb[k][:], in_=w[k * 128:(k + 1) * 128, :]
                               ).then_inc(w_sem[k], 16)
            sync.dma_start(out=b_sb[:], in_=bb[:]).then_inc(bb_sem, 16)
            for i, (tb, half) in enumerate(CLOSES):
                r = slice(tb * 128, (tb + 1) * 128)
                cols = slice(0, 512) if half == 0 else slice(512, 768)
                sync.wait_ge(cp_sem, i + 1)
                sync.dma_start(out=out[r, cols],
                               in_=o_sb[tb][:, cols]).then_inc(out_sem, 16)
            sync.wait_ge(out_sem, 16 * 2 * TB)

        # ACT HWDGE ring: x chunks in.  x0 is split: the tb0/tb1 token
        # columns (0:256) land first to unblock the PE; tb2/tb3's columns
        # follow (only needed by the backfill much later).
        @block.scalar
        def _(scalar):
            scalar.dma_start(out=x_sb[0][:, 0:256], in_=xT[0:128, 0:256]
                             ).then_inc(x_sem[0], 16)
            scalar.dma_start(out=x_sb[0][:, 256:512], in_=xT[0:128, 256:512]
                             ).then_inc(x0b_sem, 16)
            for k in range(1, KT):
                scalar.dma_start(out=x_sb[k][:], in_=xT[k * 128:(k + 1) * 128, :]
                                 ).then_inc(x_sem[k], 16)

        @block.tensor
        def _(tensor):
            # tb0/tb1 chase the arriving chunks and close early; tb2/tb3
            # backfill afterwards (all chunks resident by then).  Within a
            # block the a-half (cols 0:512) chain runs before the b-half so
            # the halves close staggered, per CLOSES order.
            def mm(tb, half, k):
                lhsT = x_sb[k][:, tb * 128:(tb + 1) * 128]
                if half == 0:
                    m = tensor.matmul(ps_a[tb][:], lhsT, w_sb[k][:, 0:512],
                                      start=(k == 0), stop=(k == KT - 1))
                else:
                    m = tensor.matmul(ps_b[tb][:, 0:256], lhsT,
                                      w_sb[k][:, 512:768],
                                      start=(k == 0), stop=(k == KT - 1))
                if k == KT - 1:
                    m.then_inc(pe_sem, 1)

            # phase 0: the small b-half matmuls run first — their inputs
            # (w0 cols 512:768, x0 cols 0:256) are the first DMAs to land.
            tensor.wait_ge(w0h_sem, 16)
            tensor.wait_ge(x_sem[0], 16)
            mm(0, 1, 0)
            mm(1, 1, 0)
            tensor.wait_ge(w_sem[0], 16)
            mm(0, 0, 0)
            mm(1, 0, 0)
            for k in range(1, KT):
                tensor.wait_ge(w_sem[k], 16)
                tensor.wait_ge(x_sem[k], 16)
                mm(0, 0, k)
                mm(1, 0, k)
                mm(0, 1, k)
                mm(1, 1, k)
            tensor.wait_ge(x0b_sem, 16)
            for tb in (2, 3):
                for k in range(KT):
                    mm(tb, 0, k)
                for k in range(KT):
                    mm(tb, 1, k)

        # DVE: bias add fused into the PSUM->SBUF (f32->bf16) copy, one op
        # per closed column group, in close order.
        @block.vector
        def _(vector):
            vector.wait_ge(bb_sem, 16)
            for i, (tb, half) in enumerate(CLOSES):
                vector.wait_ge(pe_sem, i + 1)
                if half == 0:
                    vector.tensor_add(o_sb[tb][:, 0:512], ps_a[tb][:],
                                      b_sb[:, 0:512]).then_inc(cp_sem, 1)
                else:
                    vector.tensor_add(o_sb[tb][:, 512:768], ps_b[tb][:, 0:256],
                                      b_sb[:, 512:768]).then_inc(cp_sem, 1)

    return nc


def kernel(x, W_attn, b_attn, W_proj, b_proj):
    global _nc_cache, LAST
    x = np.asarray(x, dtype=np.float32)
    W_attn = np.asarray(W_attn, dtype=np.float32)
    b_attn = np.asarray(b_attn, dtype=np.float32)
    W_proj = np.asarray(W_proj, dtype=np.float32)
    b_proj = np.asarray(b_proj, dtype=np.float32)

    # Fold the (collapsed) value + output projections into one weight.
    W_fused = W_attn[:, 2 * E:3 * E] @ W_proj                # [768, 768]
    b_fused = b_attn[2 * E:3 * E] @ W_proj + b_proj          # [768]

    xT = np.ascontiguousarray(x.reshape(TOKENS, E).T)        # [768, 4096]
    xT_bf = xT.astype(ml_dtypes.bfloat16)
    w_bf = W_fused.astype(ml_dtypes.bfloat16)
    bb_bf = np.ascontiguousarray(
        np.broadcast_to(b_fused.astype(ml_dtypes.bfloat16), (128, E)))

    if _nc_cache is None:
        _nc_cache = _build()
    nc = _nc_cache

    in_maps = [
        {
            "xT": np.ascontiguousarray(xT_bf[:, c * TPC:(c + 1) * TPC]),
            "w": w_bf,
            "bb": bb_bf,
        }
        for c in range(N_CORES)
    ]
    # The axon-tunneled devices occasionally come up in an unrecoverable
    # state from a previous session; a short backoff and retry clears it.
    import time
    for attempt in range(3):
        try:
            res = run_bass_kernel_spmd(nc, in_maps,
                                       core_ids=list(range(N_CORES)),
                                       trace=TRACE)
            break
        except Exception:
            if attempt == 2:
                raise
            time.sleep(15 * (attempt + 1))
    LAST = res
    out = np.concatenate([res.results[c]["out"] for c in range(N_CORES)], axis=0)
    return out.reshape(B, S, E).astype(np.float32)


# revision 44
# speedup vs baseline: 1.1946x; 1.1946x over previous
"""Trainium2 Bass kernel for nn_Attention_21208548508357.

Math note: the reference module's einsum is `'bhij,bihd->bihd'` — the value
tensor is indexed with the *query* position `i`, so `j` (the key position)
appears only in the softmax matrix. The einsum therefore reduces to
`v[b,i,h,d] * sum_j att[b,h,i,j]`, and softmax rows sum to exactly 1, so the
whole attention block is the identity on `v`:

    out = (x @ W_v + b_v) @ W_proj + b_proj
        = x @ (W_v @ W_proj) + (b_v @ W_proj + b_proj)

where W_v = W_attn[:, 2E:3E], b_v = b_attn[2E:3E].  The device kernel runs
the token-sharded GEMM `out = x @ W_fused + b_fused` SPMD on 8 NeuronCores
(512 tokens per core), with the tiny 768x768 weight-fold done on host.

Device layout (per core):
  xT  [768, 512]  bf16  — x shard transposed (contraction dim on partitions)
  w   [768, 768]  bf16  — fused weight
  bb  [128, 768]  bf16  — fused bias broadcast to all partitions
  out [512, 768]  bf16  — host upcasts to f32

Structure: PE stationary = 128x128 xT tile, moving = w rows, fp32 PSUM
accumulate over 6 k-tiles; column split 512+256 along PSUM banks.  Weights
and outputs ride the SP HWDGE ring, x chunks the ACT ring.  Token blocks
0/1 chase the arriving chunks and close early, 2/3 backfill, so the DVE
bias-add (fused with the f32->bf16 PSUM->SBUF copy) and the output DMAs
overlap the tail of the matmul stream.  Raw bass (no Tile) — each wait is a
standalone InstEventSemaphore since this walrus build rejects multi-wait
instructions, and every DMA chunk gets its own semaphore so no DMA
completion-order assumptions are needed.
"""

import numpy as np
import sys

if "/opt/trn_rl_repo" not in sys.path:
    sys.path.insert(0, "/opt/trn_rl_repo")

import ml_dtypes
import concourse.bass as bass
import concourse.mybir as mybir
from concourse.bass_utils import run_bass_kernel_spmd

N_CORES = 8
B, S, E = 2, 2048, 768
TOKENS = B * S                    # 4096
TPC = TOKENS // N_CORES           # 512 tokens per core
KT = E // 128                     # 6 contraction tiles of 128
TB = TPC // 128                   # 4 token blocks of 128 per core

BF16 = mybir.dt.bfloat16
F32 = mybir.dt.float32

TRACE = False      # test.py flips this to profile
LAST = None        # last BassKernelResults when TRACE

_nc_cache = None


def _build():
    nc = bass.Bass()
    xT = nc.declare_dram_parameter("xT", [E, TPC], BF16, isOutput=False)
    w = nc.declare_dram_parameter("w", [E, E], BF16, isOutput=False)
    bb = nc.declare_dram_parameter("bb", [128, E], BF16, isOutput=False)
    # token-row indices for the scatter-writeback of tb2/tb3 (int16,
    # wrapped in 16 partitions: idx j of block t lives at [j%16, t*8 + j//16])
    idx = nc.declare_dram_parameter("idx", [128, 16], mybir.dt.int16,
                                    isOutput=False)
    out = nc.declare_dram_parameter("out", [TPC, E], BF16, isOutput=True)

    with bass.ExitStack() as ctx:
        w_sb = [ctx.enter_context(nc.sbuf_tensor(f"w_sb{k}", [128, E], BF16))
                for k in range(KT)]
        x_sb = [ctx.enter_context(nc.sbuf_tensor(f"x_sb{k}", [128, TPC], BF16))
                for k in range(KT)]
        b_sb = ctx.enter_context(nc.sbuf_tensor("b_sb", [128, E], BF16))
        idx_sb = ctx.enter_context(nc.sbuf_tensor("idx_sb", [128, 16],
                                                  mybir.dt.int16))
        z_sb = ctx.enter_context(nc.sbuf_tensor("z_sb", [128, E], BF16))
        o_sb = [ctx.enter_context(nc.sbuf_tensor(f"o_sb{t}", [128, E], BF16))
                for t in range(TB)]
        # one PSUM bank (2KB) per tensor: a = f[0:512], b = f[512:768]
        ps_a = [ctx.enter_context(nc.psum_tensor(f"ps_a{t}", [128, 512], F32))
                for t in range(TB)]
        ps_b = [ctx.enter_context(nc.psum_tensor(f"ps_b{t}", [128, 512], F32))
                for t in range(TB)]

        w_sem = [ctx.enter_context(nc.semaphore(f"w_sem{k}")) for k in range(KT)]
        w0h_sem = ctx.enter_context(nc.semaphore("w0h_sem"))
        x0b_sem = ctx.enter_context(nc.semaphore("x0b_sem"))
        x_sem = [ctx.enter_context(nc.semaphore(f"x_sem{k}")) for k in range(KT)]
        bb_sem = ctx.enter_context(nc.semaphore("bb_sem"))
        pe_sem = ctx.enter_context(nc.semaphore("pe_sem"))
        cp_sem = ctx.enter_context(nc.semaphore("cp_sem"))
        out_sem = ctx.enter_context(nc.semaphore("out_sem"))
        pidx_sem = ctx.enter_context(nc.semaphore("pidx_sem"))
        prep_sem = ctx.enter_context(nc.semaphore("prep_sem"))
        sout_sem = ctx.enter_context(nc.semaphore("sout_sem"))
        zs_sem = ctx.enter_context(nc.semaphore("zs_sem"))
        zd_sem = ctx.enter_context(nc.semaphore("zd_sem"))
        block = ctx.enter_context(nc.Block())

        # Column-group close order: (tb, half).  Each entry closes its fp32
        # accumulation independently; the DVE bias-add and output DMA for a
        # group run while later groups are still accumulating on the PE.
        CLOSES = [(0, 0), (1, 0), (0, 1), (1, 1), (2, 0), (2, 1), (3, 0), (3, 1)]

        # SP HWDGE ring: weights (w0 split in half so the first matmul can
        # start sooner), the broadcast bias, then the output DMAs (SP's DGE
        # latency is lower than ACT's and its ring is idle by output time).
        @block.sync
        def _(sync):
            sync.dma_start(out=w_sb[0][:, 512:768], in_=w[0:128, 512:768]
                           ).then_inc(w0h_sem, 16)
            sync.dma_start(out=w_sb[0][:, 0:512], in_=w[0:128, 0:512]
                           ).then_inc(w_sem[0], 16)
            for k in range(1, KT):
                sync.dma_start(out=w_sb[k][:], in_=w[k * 128:(k + 1) * 128, :]
                               ).then_inc(w_sem[k], 16)
            sync.dma_start(out=b_sb[:], in_=bb[:]).then_inc(bb_sem, 16)
            for i, (tb, half) in enumerate(CLOSES[:4]):
                r = slice(tb * 128, (tb + 1) * 128)
                cols = slice(0, 512) if half == 0 else slice(512, 768)
                sync.wait_ge(cp_sem, i + 1)
                sync.dma_start(out=out[r, cols],
                               in_=o_sb[tb][:, cols]).then_inc(out_sem, 16)
            sync.wait_ge(out_sem, 16 * 4)

        # Pool/SWDGE: the last four output pieces (tb2/tb3) go through the
        # prepare+trigger path — descriptors are generated ahead of time on
        # the otherwise-idle Q7, so once the DVE finishes a piece only a
        # cheap trigger + the transfer itself remain (the plain HWDGE path
        # pays its full issue latency after the data is ready).  The scatter
        # accumulates onto the output buffer, which both run paths pre-zero.
        @block.gpsimd
        def _(gpsimd):
            from concourse import library_config
            gpsimd.load_library(library_config.mlp)
            gpsimd.dma_start(out=idx_sb[:], in_=idx[:]).then_inc(pidx_sem, 16)
            gpsimd.wait_ge(pidx_sem, 16)
            for i, (tb, half) in enumerate(CLOSES[4:]):
                cols = slice(0, 512) if half == 0 else slice(512, 768)
                nel = cols.stop - cols.start
                in3 = o_sb[tb][:, cols].rearrange("p (o e) -> p o e", o=1)
                gpsimd.dma_scatter_add(
                    out_ap=out[:, cols], in_ap=in3,
                    idxs_ap=idx_sb[:, (tb - 2) * 8:(tb - 1) * 8],
                    num_idxs=128, num_idxs_reg=128,
                    elem_size=nel, elem_step=E,
                    prepare_only=True, sem=sout_sem,
                ).then_inc(prep_sem, 1)
            gpsimd.wait_ge(zd_sem, 32)
            for i in range(4):
                gpsimd.wait_ge(prep_sem, i + 1)
                gpsimd.wait_ge(cp_sem, 4 + i + 1)
                gpsimd.trigger_dma(count=1)
            gpsimd.wait_ge(sout_sem, 16 * 4)

        # ACT HWDGE ring: x chunks in.  x0 is split: the tb0/tb1 token
        # columns (0:256) land first to unblock the PE; tb2/tb3's columns
        # follow (only needed by the backfill much later).
        @block.scalar
        def _(scalar):
            scalar.dma_start(out=x_sb[0][:, 0:256], in_=xT[0:128, 0:256]
                             ).then_inc(x_sem[0], 16)
            scalar.dma_start(out=x_sb[0][:, 256:512], in_=xT[0:128, 256:512]
                             ).then_inc(x0b_sem, 16)
            for k in range(1, KT):
                scalar.dma_start(out=x_sb[k][:], in_=xT[k * 128:(k + 1) * 128, :]
                                 ).then_inc(x_sem[k], 16)
            # pre-zero the tb2/tb3 output rows (the scatter-writeback path
            # accumulates) — runs long before the scatters fire.
            scalar.wait_ge(zs_sem, 1)
            scalar.dma_start(out=out[256:384, :], in_=z_sb[:]).then_inc(zd_sem, 16)
            scalar.dma_start(out=out[384:512, :], in_=z_sb[:]).then_inc(zd_sem, 16)

        @block.tensor
        def _(tensor):
            # tb0/tb1 chase the arriving chunks and close early; tb2/tb3
            # backfill afterwards (all chunks resident by then).  Within a
            # block the a-half (cols 0:512) chain runs before the b-half so
            # the halves close staggered, per CLOSES order.
            def mm(tb, half, k):
                lhsT = x_sb[k][:, tb * 128:(tb + 1) * 128]
                if half == 0:
                    m = tensor.matmul(ps_a[tb][:], lhsT, w_sb[k][:, 0:512],
                                      start=(k == 0), stop=(k == KT - 1))
                else:
                    m = tensor.matmul(ps_b[tb][:, 0:256], lhsT,
                                      w_sb[k][:, 512:768],
                                      start=(k == 0), stop=(k == KT - 1))
                if k == KT - 1:
                    m.then_inc(pe_sem, 1)

            # phase 0: the small b-half matmuls run first — their inputs
            # (w0 cols 512:768, x0 cols 0:256) are the first DMAs to land.
            tensor.wait_ge(w0h_sem, 16)
            tensor.wait_ge(x_sem[0], 16)
            mm(0, 1, 0)
            mm(1, 1, 0)
            tensor.wait_ge(w_sem[0], 16)
            mm(0, 0, 0)
            mm(1, 0, 0)
            for k in range(1, KT):
                tensor.wait_ge(w_sem[k], 16)
                tensor.wait_ge(x_sem[k], 16)
                mm(0, 0, k)
                mm(1, 0, k)
                mm(0, 1, k)
                mm(1, 1, k)
            tensor.wait_ge(x0b_sem, 16)
            for tb in (2, 3):
                for k in range(KT):
                    mm(tb, 0, k)
                for k in range(KT):
                    mm(tb, 1, k)

        # DVE: bias add fused into the PSUM->SBUF (f32->bf16) copy, one op
        # per closed column group, in close order.
        @block.vector
        def _(vector):
            vector.memset(z_sb[:], 0.0).then_inc(zs_sem, 1)
            vector.wait_ge(bb_sem, 16)
            for i, (tb, half) in enumerate(CLOSES):
                vector.wait_ge(pe_sem, i + 1)
                if half == 0:
                    vector.tensor_add(o_sb[tb][:, 0:512], ps_a[tb][:],
                                      b_sb[:, 0:512]).then_inc(cp_sem, 1)
                else:
                    vector.tensor_add(o_sb[tb][:, 512:768], ps_b[tb][:, 0:256],
                                      b_sb[:, 512:768]).then_inc(cp_sem, 1)

    # Raw bass skips Bacc's codegen_inst_isa_subclasses pass; without it the
    # extended Pool instructions (library reload, scatter prep, trigger)
    # reach walrus with empty .instr bytes -> "ISA wrong length".
    from concourse.library_overlay import lower_extended_insts
    lower_extended_insts(nc)
    return nc


def kernel(x, W_attn, b_attn, W_proj, b_proj):
    global _nc_cache, LAST
    x = np.asarray(x, dtype=np.float32)
    W_attn = np.asarray(W_attn, dtype=np.float32)
    b_attn = np.asarray(b_attn, dtype=np.float32)
    W_proj = np.asarray(W_proj, dtype=np.float32)
    b_proj = np.asarray(b_proj, dtype=np.float32)

    # Fold the (collapsed) value + output projections into one weight.
    W_fused = W_attn[:, 2 * E:3 * E] @ W_proj                # [768, 768]
    b_fused = b_attn[2 * E:3 * E] @ W_proj + b_proj          # [768]

    xT = np.ascontiguousarray(x.reshape(TOKENS, E).T)        # [768, 4096]
    xT_bf = xT.astype(ml_dtypes.bfloat16)
    w_bf = W_fused.astype(ml_dtypes.bfloat16)
    bb_bf = np.ascontiguousarray(
        np.broadcast_to(b_fused.astype(ml_dtypes.bfloat16), (128, E)))

    # scatter indices: block t's idx j (= local token row 128*(t+2)+j) sits
    # at [j % 16, t*8 + j // 16]; rows 16..127 replicate rows 0..15.
    idx_np = np.zeros((16, 16), np.int16)
    for t in range(2):
        for j in range(128):
            idx_np[j % 16, t * 8 + j // 16] = 128 * (t + 2) + j
    idx_np = np.ascontiguousarray(np.tile(idx_np, (8, 1)))

    if _nc_cache is None:
        _nc_cache = _build()
    nc = _nc_cache

    in_maps = [
        {
            "xT": np.ascontiguousarray(xT_bf[:, c * TPC:(c + 1) * TPC]),
            "w": w_bf,
            "bb": bb_bf,
            "idx": idx_np,
        }
        for c in range(N_CORES)
    ]
    # The axon-tunneled devices occasionally come up in an unrecoverable
    # state from a previous session; a short backoff and retry clears it.
    import time
    for attempt in range(3):
        try:
            res = run_bass_kernel_spmd(nc, in_maps,
                                       core_ids=list(range(N_CORES)),
                                       trace=TRACE)
            break
        except Exception:
            if attempt == 2:
                raise
            time.sleep(15 * (attempt + 1))
    LAST = res
    out = np.concatenate([res.results[c]["out"] for c in range(N_CORES)], axis=0)
    return out.reshape(B, S, E).astype(np.float32)
